# revision 32
# baseline (speedup 1.0000x reference)
"""Trainium2 Bass kernel for efficient-attention (nn_Attention_65532611003000).

Sharding: data-parallel over batch. B == n_cores == 8, so core i processes
batch element i end-to-end; no collectives are needed.

Layout strategy: x and y are pre-transposed on the host to channel-major
chunks, so the kernel needs ZERO PE transposes (the previous version spent
~half its tensor-engine time on 384 128x128 transposes).

Per-core math ([Nt, Ch] = [4096, 512] activations, H=8 heads, 64 ch/head):
  pass 1 (per 128-token chunk, contraction over channel blocks t):
    xsT  = xT + yT                        # channel-major, DVE
    kpre[tok,:] = sum_t xsT_t^T @ Wk_t    # bk drops out (token softmax)
    khat = exp(kpre)                      # bf16
    vpre[tok,:] = sum_t xT_t^T @ Wv_t
    S_t += khat_t^T @ [vpre_t | 1]        # ones col accumulates Zk
  epilogue:
    ctx  = S / Zk + bv                    # per head: [64, 64] blockdiag
  pass 2 (per group of 4 chunks = 512 tokens, channel-major throughout):
    qpreT[s] = sum_t Wq[t,s]^T @ yT_t     # [128 kch, 512 tok]
    qhatT[s] = exp(qpreT[s] + bq[s])      # per-partition bias on Act engine
    Z[h,tok] = sum_s onesblk_s^T @ qhatT[s]   # partition-group sums via PE
    attT_raw[s] = ctxR_s^T @ qhatT[s]     # [128 vch, 512 tok]
    zb[s]    = bcast8_s^T @ (1/Z)         # broadcast normalizer to vch rows
    attn[s]  = attT_raw[s] * zb[s]        # DVE, fused into PSUM->SBUF copy
    opre[j]  = sum_s attn[s][:,j]^T @ Wr_s ; out = opre + br  (one group late)
"""

import sys

sys.path.insert(0, "/opt/trn_rl_repo")

import numpy as np
import ml_dtypes
from contextlib import ExitStack

import concourse.bass as bass
import concourse.bacc as bacc
import concourse.mybir as mybir
import concourse.tile as tile
from concourse.bass_utils import run_bass_kernel_spmd

B, Nt, Ch = 8, 4096, 512
H, HK = 8, 64
P = 128            # token chunk rows / SBUF partitions
NT = Nt // P       # 32 token chunks
CT = Ch // P       # 4 contraction tiles
GRP = 4            # pass-2 chunks per group (512 tokens)
NG = NT // GRP

F32 = mybir.dt.float32
F32R = mybir.dt.float32r
BF16 = mybir.dt.bfloat16
AX = mybir.AxisListType
AF = mybir.ActivationFunctionType

BF16_NP = ml_dtypes.bfloat16


def build_nc(debug=False):
    nc = bacc.Bacc(None)

    xyT_d = nc.declare_dram_parameter(
        "xyT", [P, NT * 2 * CT * P], BF16, isOutput=False
    )
    wk_d = nc.declare_dram_parameter("Wk_r", [P, CT * Ch], BF16, isOutput=False)
    wv_d = nc.declare_dram_parameter("Wv_r", [P, CT * Ch], BF16, isOutput=False)
    wr_d = nc.declare_dram_parameter("Wr_r", [P, CT * Ch], BF16, isOutput=False)
    wqb_d = nc.declare_dram_parameter("Wq_b", [P, CT * CT * P], BF16, isOutput=False)
    bqc_d = nc.declare_dram_parameter("bq_col", [P, CT], F32, isOutput=False)
    brb_d = nc.declare_dram_parameter("br_bcast", [P, Ch], F32, isOutput=False)
    bvb_d = nc.declare_dram_parameter("bv_blk", [P, Ch], BF16, isOutput=False)
    obd_d = nc.declare_dram_parameter("onesbd", [P, P], BF16, isOutput=False)
    id16_d = nc.declare_dram_parameter("ident16", [P, P], BF16, isOutput=False)
    onescol_d = nc.declare_dram_parameter(
        "ones_col", [P, CT * 2], BF16, isOutput=False
    )
    out_d = nc.declare_dram_parameter("out", [Nt, Ch], F32, isOutput=True)
    if debug:
        dbg = {
            "dbg_qpreT": nc.declare_dram_parameter("dbg_qpreT", [P, CT * GRP * P], F32, isOutput=True),
            "dbg_qhatT": nc.declare_dram_parameter("dbg_qhatT", [P, CT * GRP * P], F32, isOutput=True),
            "dbg_z": nc.declare_dram_parameter("dbg_z", [8, GRP * P], F32, isOutput=True),
            "dbg_zinv": nc.declare_dram_parameter("dbg_zinv", [8, GRP * P], F32, isOutput=True),
            "dbg_zb": nc.declare_dram_parameter("dbg_zb", [P, CT * GRP * P], F32, isOutput=True),
            "dbg_attn": nc.declare_dram_parameter("dbg_attn", [P, CT * GRP * P], F32, isOutput=True),
            "dbg_ctx": nc.declare_dram_parameter("dbg_ctx", [P, CT * P], F32, isOutput=True),
        }

    with tile.TileContext(nc) as tc, ExitStack() as ctx:
        const = ctx.enter_context(tc.tile_pool(name="const", bufs=1))

        wk = const.tile([P, CT, Ch], BF16)
        wv = const.tile([P, CT, Ch], BF16)
        wr = const.tile([P, CT, Ch], BF16)
        wqb = const.tile([P, CT, CT, P], BF16)
        bqc = const.tile([P, CT], F32)
        brb = const.tile([P, Ch], F32)
        bvb = const.tile([P, Ch], BF16)
        obd = const.tile([P, P], BF16)
        id16 = const.tile([P, P], BF16)
        cw = const.tile([P, CT, Ch], BF16)            # ctx @ Wr, built once
        xyT_all = const.tile([P, NT, 2, CT, P], BF16)  # resident x^T,y^T 64KB/part
        ctxR = const.tile([P, CT, P], BF16)           # per-head ctx, blockdiag
        zkinv = const.tile([P, CT], F32)

        for t in range(CT):
            nc.gpsimd.dma_start(
                wv[:, t, :], wv_d[:, Ch * t : Ch * (t + 1)]
            )
        for t in range(CT):
            nc.gpsimd.dma_start(
                wk[:, t, :], wk_d[:, Ch * t : Ch * (t + 1)]
            )

        xyT_v = xyT_d[:].rearrange(
            "p (i c t q) -> p i c t q", i=NT, c=2, t=CT
        )

        # ---------------- pass 1: khat, v, S & Zk accumulation --------------
        with (
            tc.tile_pool(name="io1", bufs=4) as io1,
            tc.tile_pool(name="sb1", bufs=2) as sb1,
            tc.tile_pool(name="ps_k", bufs=2, space="PSUM") as ps_k,
            tc.tile_pool(name="ps_v", bufs=2, space="PSUM") as ps_v,
            tc.tile_pool(name="ps_s", bufs=1, space="PSUM") as ps_s,
        ):
            s_acc = [
                ps_s.tile([P, 130], F32, tag=f"sacc{t}", name=f"sacc{t}")
                for t in range(CT)
            ]
            # manually double-buffered [val0 | ones] tiles; ones cols written once
            v_aug_bufs = [
                sb1.tile([P, CT, 130], BF16, tag=f"vaug{n}", name=f"vaug{n}")
                for n in range(2)
            ]
            for n in range(2):
                nc.gpsimd.dma_start(
                    v_aug_bufs[n][:, :, 128:130],
                    onescol_d[:].rearrange("p (t c) -> p t c", t=CT),
                )

            for i in range(NT):
                if i < 3:
                    # split the first chunks into x/y halves: the x half
                    # unblocks vpre sooner and small transfers pipeline
                    # better through the cold DMA engines
                    nc.sync.dma_start(
                        xyT_all[:, i, 0, :, :], xyT_v[:, i, 0, :, :]
                    )
                    nc.sync.dma_start(
                        xyT_all[:, i, 1, :, :], xyT_v[:, i, 1, :, :]
                    )
                else:
                    nc.sync.dma_start(
                        xyT_all[:, i, :, :, :], xyT_v[:, i, :, :, :]
                    )
                if i == 4:
                    # pass-2/epilogue consts: issue once the pipeline is
                    # rolling so they don't contend with startup transfers
                    nc.gpsimd.dma_start(
                        wr[:], wr_d[:].rearrange("p (t j) -> p t j", t=CT)
                    )
                    nc.gpsimd.dma_start(
                        wqb[:],
                        wqb_d[:].rearrange("p (t s j) -> p t s j", t=CT, s=CT),
                    )
                    nc.gpsimd.dma_start(bqc[:], bqc_d[:])
                    nc.gpsimd.dma_start(brb[:], brb_d[:])
                    nc.gpsimd.dma_start(bvb[:], bvb_d[:])
                    nc.gpsimd.dma_start(obd[:], obd_d[:])
                    nc.gpsimd.dma_start(id16[:], id16_d[:])

                xsT = sb1.tile([P, CT, P], BF16, tag="xsT")
                nc.vector.tensor_add(
                    xsT[:], xyT_all[:, i, 0, :, :], xyT_all[:, i, 1, :, :]
                )

                vpre = ps_v.tile([P, Ch], F32, tag="vpre")
                for t in range(CT):
                    nc.tensor.matmul(
                        vpre[:],
                        xyT_all[:, i, 0, t, :],
                        wv[:, t, :],
                        start=(t == 0),
                        stop=(t == CT - 1),
                    )
                v_aug = v_aug_bufs[i % 2]
                nc.scalar.copy(
                    v_aug[:, :, 0:128],
                    vpre[:].rearrange("p (t q) -> p t q", t=CT),
                )

                kpre = ps_k.tile([P, Ch], F32, tag="kpre")
                for t in range(CT):
                    nc.tensor.matmul(
                        kpre[:],
                        xsT[:, t, :],
                        wk[:, t, :],
                        start=(t == 0),
                        stop=(t == CT - 1),
                    )
                khat = sb1.tile([P, Ch], BF16, tag="khat")
                nc.scalar.activation(khat[:], kpre[:], AF.Exp)

                for t in range(CT):
                    nc.tensor.matmul(
                        s_acc[t][:],
                        khat[:, P * t : P * (t + 1)],
                        v_aug[:, t, :],
                        start=(i == 0),
                        stop=(i == NT - 1),
                    )

            # ------------- epilogue: ctx = S * zkinv + bv ------------------
            for t in range(CT):
                nc.vector.reciprocal(zkinv[:, t : t + 1], s_acc[t][:, 128:129])
            for t in range(CT):
                nc.vector.tensor_copy(ctxR[:, t, :], bvb[:, P * t : P * (t + 1)])
                for blk in range(2):
                    p0 = 64 * blk
                    nc.vector.scalar_tensor_tensor(
                        ctxR[p0 : p0 + 64, t, p0 : p0 + 64],
                        s_acc[t][p0 : p0 + 64, p0 : p0 + 64],
                        zkinv[p0 : p0 + 64, t : t + 1],
                        bvb[p0 : p0 + 64, P * t + p0 : P * t + p0 + 64],
                        op0=mybir.AluOpType.mult,
                        op1=mybir.AluOpType.add,
                    )
            if debug:
                ctx_dump = sb1.tile([P, CT, P], F32, name="ctx_dump", tag="ctxd")
                nc.vector.tensor_copy(ctx_dump[:], ctxR[:])
                nc.sync.dma_start(
                    dbg["dbg_ctx"][:].rearrange("p (t c) -> p t c", t=CT),
                    ctx_dump[:],
                )

        # ---------------- pass 2: q softmax, attend, reproject ---------------
        # Emission order per group g: qpre(g), z(g), attT(g), zb(g),
        # mults(g) on DVE, then opre(g-1) — the reprojection runs one group
        # late so the PE never stalls waiting for the DVE multiply chain.
        with (
            tc.tile_pool(name="io2", bufs=4) as io2,
            tc.tile_pool(name="sb2", bufs=2) as sb2,
            tc.tile_pool(name="ps_q", bufs=3, space="PSUM") as ps_q,
            tc.tile_pool(name="ps_zb", bufs=2, space="PSUM") as ps_zb,
            tc.tile_pool(name="ps_o", bufs=2, space="PSUM") as ps_o,
        ):
            qn_bufs = [
                sb2.tile([P, CT, GRP * P], BF16, tag=f"qn{n}", name=f"qn{n}")
                for n in range(2)
            ]

            def emit_opre(g):
                qn = qn_bufs[g % 2]
                for j in range(GRP):
                    i = g * GRP + j
                    opre = ps_o.tile([P, Ch], F32, tag="opre")
                    for s in range(CT):
                        nc.tensor.matmul(
                            opre[:],
                            qn[:, s, P * j : P * (j + 1)],
                            cw[:, s, :],
                            start=(s == 0),
                            stop=(s == CT - 1),
                        )
                    o_sb = io2.tile([P, Ch], F32, tag="osb")
                    if g == NG - 1 and j % 2 == 1:
                        nc.vector.tensor_copy(o_sb[:], opre[:])
                    else:
                        nc.scalar.copy(o_sb[:], opre[:])
                    nc.sync.dma_start(out_d[P * i : P * (i + 1), :], o_sb[:])

            for g in range(NG):
                qhatT = sb2.tile([P, CT, GRP * P], BF16, tag="qhatT")
                zbinv = sb2.tile([P, CT, GRP * P], F32, tag="zbinv")
                for s in range(CT):
                    qpre = ps_q.tile([P, GRP * P], F32, tag="qpre")
                    for t in range(CT):
                        nc.tensor.matmul(
                            qpre[:],
                            wqb[:, t, s, :],
                            xyT_all[:, GRP * g : GRP * (g + 1), 1, t, :],
                            start=(t == 0),
                            stop=(t == CT - 1),
                        )
                    nc.scalar.activation(
                        qhatT[:, s, :], qpre[:], AF.Exp, bias=bqc[:, s : s + 1]
                    )
                    # zb[p,tok] = sum_{k in head(p)} qhatT[k,tok]: the ones
                    # blockdiag stationary lands Z pre-broadcast on all 128
                    # partitions; feeds opre(g) one group later, so the
                    # reciprocal never gates the PE.
                    zb = ps_zb.tile([P, GRP * P], F32, tag="zb")
                    nc.tensor.matmul(
                        zb[:], obd[:], qhatT[:, s, :], start=True, stop=True
                    )
                    nc.vector.reciprocal_approx_fast(zbinv[:, s, :], zb[:])
                    if g == 0 and s == 1:
                        # CW[k,c] = sum_v ctx[k,v] Wr[v,c] + br_eff/8 per row
                        # (head softmax sums to 1). Emitted mid-group so the
                        # ctx-epilogue wait overlaps qpre matmuls and the
                        # ctxT copies land early in the Act queue.
                        for t in range(CT):
                            ctxT_ps = ps_zb.tile([P, P], BF16, tag="zb")
                            nc.tensor.transpose(
                                ctxT_ps[:], ctxR[:, t, :], id16[:]
                            )
                            ctxT = sb2.tile([P, P], BF16, tag="ctxTs")
                            nc.scalar.copy(ctxT[:], ctxT_ps[:])
                            cw_ps = ps_q.tile([P, Ch], F32, tag="qpre")
                            nc.tensor.matmul(
                                cw_ps[:], ctxT[:], wr[:, t, :],
                                start=True, stop=True,
                            )
                            nc.vector.tensor_add(cw[:, t, :], cw_ps[:], brb[:])
                    if debug and g == 0:
                        nc.sync.dma_start(
                            dbg["dbg_qpreT"][:, GRP * P * s : GRP * P * (s + 1)],
                            qpre[:],
                        )
                        nc.sync.dma_start(
                            dbg["dbg_zb"][:, GRP * P * s : GRP * P * (s + 1)],
                            zbinv[:, s, :],
                        )

                if g > 0:
                    emit_opre(g - 1)

                qn = qn_bufs[g % 2]
                for s in range(CT):
                    nc.vector.tensor_mul(
                        qn[:, s, :], qhatT[:, s, :], zbinv[:, s, :]
                    )

                if debug and g == 0:
                    qh_dump = sb2.tile([P, CT, GRP * P], F32, tag="qhd", name="qhd")
                    nc.vector.tensor_copy(qh_dump[:], qhatT[:])
                    nc.sync.dma_start(
                        dbg["dbg_qhatT"][:].rearrange("p (s q) -> p s q", s=CT),
                        qh_dump[:],
                    )
                    z_dump = sb2.tile([8, GRP * P], F32, tag="zd", name="zd")
                    nc.vector.tensor_copy(z_dump[:], z_ps[:])
                    nc.sync.dma_start(dbg["dbg_z"][:], z_dump[:])
                    zi_dump = sb2.tile([8, GRP * P], F32, tag="zid", name="zid")
                    nc.vector.tensor_copy(zi_dump[:], zinv[:])
                    nc.sync.dma_start(dbg["dbg_zinv"][:], zi_dump[:])
                    nc.sync.dma_start(
                        dbg["dbg_attn"][:].rearrange("p (s q) -> p s q", s=CT),
                        attn[:],
                    )

            emit_opre(NG - 1)

    nc.finalize()
    return nc


def _host_consts(Wk, bk, Wq, bq, Wv, bv, Wr, br):
    def rearr(w):
        return (
            np.ascontiguousarray(
                w.reshape(CT, P, Ch).transpose(1, 0, 2).reshape(P, CT * Ch)
            ).astype(BF16_NP)
        )

    # Wq in [ch-part, t, s, kch] block form
    wqb = np.ascontiguousarray(
        Wq.reshape(CT, P, CT, P).transpose(1, 0, 2, 3).reshape(P, CT * CT * P)
    ).astype(BF16_NP)

    # ctx is built WITHOUT bv (bv @ Wr is absorbed into br below, exact
    # because the query softmax weights sum to 1); bvb is all zeros so the
    # epilogue's fused multiply-add writes S*zkinv on the diagonal blocks.
    bvb = np.zeros((P, Ch), np.float32)
    br_eff = (
        br.astype(np.float64) + bv.astype(np.float64) @ Wr.astype(np.float64)
    ).astype(np.float32)

    # blockdiag ones: obd[k, p] = 1 where (k >= 64) == (p >= 64)
    obd = np.zeros((P, P), np.float32)
    obd[0:64, 0:64] = 1.0
    obd[64:128, 64:128] = 1.0
    return {
        "Wk_r": rearr(Wk),
        "Wv_r": rearr(Wv),
        "Wr_r": rearr(Wr),
        "Wq_b": wqb,
        "bq_col": np.ascontiguousarray(
            bq.reshape(CT, P).T
        ).astype(np.float32),
        "br_bcast": np.ascontiguousarray(
            np.tile(br_eff[None, :] / 8.0, (P, 1))
        ).astype(np.float32),
        "bv_blk": bvb.astype(BF16_NP),
        "onesbd": obd.astype(BF16_NP),
        "ident16": np.eye(P).astype(BF16_NP),
        "ones_col": np.ones((P, CT * 2), BF16_NP),
    }


def _pack_xy(x, y):
    """Pack x^T,y^T chunk-interleaved: (p, i, c, t, q) = {x,y}[i*128+q, t*128+p]."""
    xc = x.reshape(NT, P, CT, P).transpose(3, 0, 2, 1)
    yc = y.reshape(NT, P, CT, P).transpose(3, 0, 2, 1)
    return (
        np.stack([xc, yc], axis=2).astype(BF16_NP).reshape(P, NT * 2 * CT * P)
    )


_NC_CACHE = {}


def _get_nc():
    if "nc" not in _NC_CACHE:
        _NC_CACHE["nc"] = build_nc()
    return _NC_CACHE["nc"]


def kernel(input_, y, Wk, bk, Wq, bq, Wv, bv, Wr, br, _trace=False, _tmpdir=None):
    input_ = np.asarray(input_, np.float32)
    y = np.asarray(y, np.float32)
    consts = _host_consts(
        np.asarray(Wk, np.float32), np.asarray(bk, np.float32),
        np.asarray(Wq, np.float32), np.asarray(bq, np.float32),
        np.asarray(Wv, np.float32), np.asarray(bv, np.float32),
        np.asarray(Wr, np.float32), np.asarray(br, np.float32),
    )
    nc = _get_nc()
    in_maps = [
        {
            "xyT": _pack_xy(input_[i], y[i]),
            **consts,
        }
        for i in range(B)
    ]
    res = run_bass_kernel_spmd(
        nc, in_maps, core_ids=list(range(B)), trace=_trace, tmpdir=_tmpdir
    )
    out = np.stack([res.results[i]["out"] for i in range(B)], axis=0)
    if _trace:
        return out, res
    return out
